# revision 41
# baseline (speedup 1.0000x reference)
"""AttentionBlock kernel for 8 Trainium2 NeuronCores.

Reference computation (per batch element b of 8):
    xn  = GroupNorm(x, 32 groups, eps=1e-5) * gn_scale + gn_bias
    qkv = w_qkv @ xn + b_qkv          (1x1 conv == channel matmul)
    q, k, v = split(qkv)              each (C=256, N=4096)
    S   = (q^T k) * C^-0.5            (N, N) scores
    A   = softmax(S, axis=-1)
    AO  = (A @ v^T)^T                 (C, N)
    out = w_out @ AO + b_out + x

Sharding: data-parallel over batch - core i computes batch element i.

Design notes:
  - HOST-SIDE FOLDS: w_out is folded into the v projection (wvo = w_out @
    w_qkv_v), so the attention AO matmul directly produces the final
    projected output, transposed: out^T = exp(S^T)^T @ [vtw | 1] / den.
    No per-block PE transposes, no per-block w_out matmuls. Residual and
    biases are folded into a host input xtb = x^T + b_out + bvo. Weights
    arrive pre-transposed from the host. The attention scale 1/16 is split
    sqrt-wise onto q and k weights (better fp8 range than all-on-q).
  - Projections (q/k/vtw) run in bf16 (fp8 error there doubles the final
    error); q3/k3 are quantized to fp8 after. S and AO matmuls are fp8
    DoubleRow (contraction 256 paired per PE cell).
  - GroupNorm stats per channel via bn_stats/bn_aggr; cross-partition group
    aggregation via tiny selector matmuls on the PE; xn evicted to bf16.
  - exp softmax skips max-subtraction (scores ~N(0,1)); exp is scaled by
    1/32 (ESC_BIAS) into fp8e4m3 range. The ones-column on vtw yields the
    softmax denominator inside the same AO accumulation.
  - ACT (ScalarE) is the critical engine in the attention phase: it does
    exps only. All PSUM evictions go to DVE; xn eviction splits DVE/GpSimd.
  - SCHEDULE: a flat slot pipeline over all 128 S key-tile pairs. k/q/vtw
    projection matmuls are interleaved into early slots so the exp stream
    starts right after GroupNorm instead of after the whole projection
    phase. AO trails the S/exp stream by LAG pairs. PSUM: S pairs
    double-buffered (2x2 banks) + 4 AO block accumulators (4x1) = 8 banks.
  - Output is written transposed (outT [N, C]) with contiguous DMA; the
    host un-transposes when assembling the full result.
"""

import numpy as np

import concourse.bass as bass
import concourse.bacc as bacc
import concourse.mybir as mybir
import concourse.tile as tile
from concourse.bass_utils import run_bass_kernel_spmd

F32 = mybir.dt.float32
BF16 = mybir.dt.bfloat16
FP8 = mybir.dt.float8e4
ESC_BIAS = -3.4657359027997265  # ln(1/32): exp scaled into fp8e4m3 range
VPAD = 16                       # vtw free-dim pad so the DR middle step %16==0

B = 8          # batch / cores
C = 256        # channels
P = 128        # partitions
CK = C // P    # channel chunks (2)
H = W = 64
N = H * W      # 4096 spatial positions
NB = 512       # query-block width (free dim)
NBLK = N // NB  # 8 query blocks
MT = N // P    # 32 key tiles of 128
NSUB = NB // P  # 4 q-subtiles per block
G = 32         # groups
GS = C // G    # channels per group (8)
EPS = 1e-5
SCALE = float(C) ** -0.5
O_QK = 2 * C   # 512 (q and k output channels; v is folded into wvo)
BN_SUB = 512   # bn_stats subgroup width
LAG = 4        # AO trails the S/exp stream by this many key-tile pairs
AOF = C + 1    # AO matmul streamed free width (channels + den column)


_TILE_FREES = []


def _tile(tc, *args, **kwargs):
    t, free = tc.tile(*args, **kwargs)
    _TILE_FREES.append(free)  # keep persistent tiles alive (GC would release)
    return t


def build_attention_program(nc, n=N):
    """Emit the full single-core program into `nc` (one batch element)."""
    nblk = max(1, n // NB)       # query blocks
    mtn = n // P                 # key tiles
    npairs = mtn // 2
    nbsub = min(n, BN_SUB)       # bn_stats subgroup width
    GL = P // GS                 # groups per channel chunk (16)
    x_d = nc.dram_tensor("x", [C, n], F32, kind="ExternalInput").ap()
    xtb_d = nc.dram_tensor("xtb", [n, C], F32, kind="ExternalInput").ap()
    wqkT_d = nc.dram_tensor("wqkT", [P, CK, O_QK], F32, kind="ExternalInput").ap()
    wvoT_d = nc.dram_tensor("wvoT", [P, CK, C], F32, kind="ExternalInput").ap()
    gns_d = nc.dram_tensor("gn_scale", [C], F32, kind="ExternalInput").ap()
    gnb_d = nc.dram_tensor("gn_bias", [C], F32, kind="ExternalInput").ap()
    bqk_d = nc.dram_tensor("bqk", [O_QK], F32, kind="ExternalInput").ap()
    out_d = nc.dram_tensor("out", [n, C], F32, kind="ExternalOutput").ap()

    from contextlib import ExitStack

    with tile.TileContext(nc) as tc, ExitStack() as ctx:
        # ---------------- persistent SBUF ----------------
        x_sb = [_tile(tc, [P, n], F32, name=f"x_sb{j}") for j in range(CK)]
        xtb_sb = _tile(tc, [P, mtn, C], F32, name="xtb_sb")
        xn_bf = _tile(tc, [P, CK, n], BF16, name="xn_bf")
        xn8 = _tile(tc, [P, CK, n], FP8, name="xn8")
        q3 = _tile(tc, [P, CK, n], FP8, name="q3")
        k3 = _tile(tc, [P, CK, n], FP8, name="k3")
        vtw_sb = _tile(tc, [P, mtn, C + VPAD], FP8, name="vtw_sb")
        expT = [_tile(tc, [P, mtn, NB], FP8, name=f"expT{v}")
                for v in range(2)]
        wqkT_f = _tile(tc, [P, CK, O_QK], F32, name="wqkT_f")
        wvoT_f = _tile(tc, [P, CK, C], F32, name="wvoT_f")
        wqkT_b = _tile(tc, [P, CK, O_QK], BF16, name="wqkT_b")
        wvoT_8 = _tile(tc, [P, CK, C], FP8, name="wvoT_8")

        # small per-channel vectors
        scale_sb = [_tile(tc, [P, 1], F32, name=f"scale_sb{j}") for j in range(CK)]
        bias_sb = [_tile(tc, [P, 1], F32, name=f"bias_sb{j}") for j in range(CK)]
        bqk_sb = [_tile(tc, [P, 1], F32, name=f"bqk_sb{i}") for i in range(2 * CK)]
        sel = [_tile(tc, [P, GL], F32, name=f"sel{j}") for j in range(CK)]
        selT = [_tile(tc, [GL, P], F32, name=f"selT{j}") for j in range(CK)]
        eps_sb = _tile(tc, [GL, 1], F32, name="eps_sb")
        eln_sb = _tile(tc, [P, 1], F32, name="eln_sb")

        # ---------------- pools (after singles: LIFO release order) -----
        ps_s = ctx.enter_context(tc.tile_pool(name="ps_s", bufs=2, space="PSUM"))
        ps_a = ctx.enter_context(tc.tile_pool(name="ps_a", bufs=4, space="PSUM"))
        work = ctx.enter_context(tc.tile_pool(name="work", bufs=3))
        evac = ctx.enter_context(tc.tile_pool(name="evac", bufs=3))

        # ---------------- input DMA ----------------
        # x first and alone on the sync queue (GN blocks on it); xtb behind
        # it on the same queue so it doesn't steal HBM bandwidth from x.
        npieces = max(1, n // BN_SUB)
        pw = n // npieces
        for j in range(CK):
            for piece in range(npieces):
                nc.sync.dma_start(
                    out=x_sb[j][:, piece * pw:(piece + 1) * pw],
                    in_=x_d[j * P:(j + 1) * P, piece * pw:(piece + 1) * pw],
                )
        # weights + small vectors on the scalar queue
        nc.scalar.dma_start(out=wqkT_f, in_=wqkT_d)
        nc.scalar.dma_start(out=wvoT_f, in_=wvoT_d)
        for j in range(CK):
            sl = slice(j * P, (j + 1) * P)
            nc.scalar.dma_start(out=scale_sb[j], in_=gns_d[sl].rearrange("(a u) -> a u", u=1))
            nc.scalar.dma_start(out=bias_sb[j], in_=gnb_d[sl].rearrange("(a u) -> a u", u=1))
        for i in range(2 * CK):
            nc.scalar.dma_start(
                out=bqk_sb[i],
                in_=bqk_d[i * P:(i + 1) * P].rearrange("(a u) -> a u", u=1),
            )
        # xtb (residual + bias fold): needed only at block finalization
        for t in range(mtn):
            nc.sync.dma_start(
                out=xtb_sb[:, t, :],
                in_=xtb_d[t * P:(t + 1) * P, :],
            )

        # ---------------- weight casts (GpSimd - keeps DVE free so
        # bn_stats can start with the first x pieces) ----------------
        nc.gpsimd.tensor_copy(wqkT_b, wqkT_f)
        nc.gpsimd.tensor_copy(wvoT_8, wvoT_f)

        # ---------------- constants ----------------
        nc.vector.memset(eps_sb, EPS)
        nc.vector.memset(eln_sb, ESC_BIAS)
        # per-chunk local selectors: sel[c, g] = 1/GS where c//GS == g
        for j in range(CK):
            nc.gpsimd.memset(sel[j], 0.0)
            nc.gpsimd.affine_select(
                out=sel[j], in_=sel[j], compare_op=mybir.AluOpType.is_gt,
                fill=1.0 / GS, base=1 - GS, pattern=[[-GS, GL]],
                channel_multiplier=1,
            )
            nc.gpsimd.affine_select(
                out=sel[j], in_=sel[j], compare_op=mybir.AluOpType.is_ge,
                fill=0.0, base=0, pattern=[[-GS, GL]], channel_multiplier=1,
            )
            nc.gpsimd.memset(selT[j], 0.0)
            nc.gpsimd.affine_select(
                out=selT[j], in_=selT[j], compare_op=mybir.AluOpType.is_gt,
                fill=1.0, base=1 - GS, pattern=[[1, P]], channel_multiplier=-GS,
            )
            nc.gpsimd.affine_select(
                out=selT[j], in_=selT[j], compare_op=mybir.AluOpType.is_ge,
                fill=0.0, base=0, pattern=[[1, P]], channel_multiplier=-GS,
            )
        # vtw ones/pad columns (den rides along the AO accumulation)
        nc.gpsimd.memset(vtw_sb[:, :, C:C + VPAD], 0.0)
        nc.gpsimd.memset(vtw_sb[:, :, C:C + 1], 1.0)

        # ---------------- group norm ----------------
        # The per-group aggregation is a chain of tiny cross-engine ops
        # whose semaphore latencies (~1us/hop) dominate. Chunk 0's chain is
        # emitted BETWEEN chunk 1's bn_stats pieces so its cross-engine
        # hops hide under the stats work on the DVE FIFO.
        npc = n // nbsub   # bn_stats pieces per chunk
        stats = [work.tile([P, npc, 6], F32, tag="bnst", name=f"bnst{j}")
                 for j in range(CK)]
        xr = [x_sb[j][:].rearrange("p (s d) -> p s d", d=nbsub)
              for j in range(CK)]
        mv2 = [work.tile([P, 2], F32, tag="mv2", name=f"mv2{j}")
               for j in range(CK)]
        # GN psum lives in ps_s (idle pre-attention) so ps_a stays free for
        # the projection phase - GN tiles released late otherwise stall the
        # k-projection bank rotation by ~25us
        ps_g = [None, None]
        ps_bc = [None, None]
        gs = [work.tile([GL, 2], F32, tag="gs", name=f"gs{j}")
              for j in range(CK)]
        gnvar = [work.tile([GL, 1], F32, tag="gvar", name=f"gvar{j}")
                 for j in range(CK)]
        gsd = [work.tile([GL, 1], F32, tag="gsd", name=f"gsd{j}")
               for j in range(CK)]
        gstat2 = [work.tile([GL, 2], F32, tag="gstat2", name=f"gstat2{j}")
                  for j in range(CK)]
        ab_c = [[None, None] for _ in range(CK)]

        def gn_chain(j, step):
            """One step of the post-stats chain for chunk j."""
            if step == 0:    # mv2 = [mean, E[x^2]]: col1 += mean^2 in place
                nc.vector.scalar_tensor_tensor(
                    out=mv2[j][:, 1:2], in0=mv2[j][:, 0:1],
                    scalar=mv2[j][:, 0:1], in1=mv2[j][:, 1:2],
                    op0=mybir.AluOpType.mult, op1=mybir.AluOpType.add)
            elif step == 1:  # group reduce on PE
                ps_g[j] = ps_s.tile([GL, 2], F32, tag="s", name=f"ps_g{j}")
                nc.tensor.matmul(ps_g[j], sel[j], mv2[j], start=True, stop=True)
            elif step == 2:
                nc.vector.tensor_copy(gs[j], ps_g[j])
            elif step == 3:  # negated group variance
                nc.vector.scalar_tensor_tensor(
                    out=gnvar[j], in0=gs[j][:, 0:1], scalar=gs[j][:, 0:1],
                    in1=gs[j][:, 1:2],
                    op0=mybir.AluOpType.mult, op1=mybir.AluOpType.subtract)
            elif step == 4:  # sqrt(var + eps)
                nc.scalar.activation(out=gsd[j], in_=gnvar[j],
                                     func=mybir.ActivationFunctionType.Sqrt,
                                     bias=eps_sb, scale=-1.0)
            elif step == 5:
                nc.vector.reciprocal(gstat2[j][:, 1:2], gsd[j])
                nc.vector.tensor_copy(gstat2[j][:, 0:1], gs[j][:, 0:1])
            elif step == 6:  # broadcast groups->channels on PE
                ps_bc[j] = ps_s.tile([P, 2], F32, tag="s", name=f"ps_bc{j}")
                nc.tensor.matmul(ps_bc[j], selT[j], gstat2[j],
                                 start=True, stop=True)
            elif step == 7:
                a_c = work.tile([P, 1], F32, tag="a_c", name=f"a_c{j}")
                nc.vector.tensor_mul(a_c, ps_bc[j][:, 1:2], scale_sb[j])
                ab_c[j][0] = a_c
            elif step == 8:
                t = work.tile([P, 1], F32, tag="t_c", name=f"t_c{j}")
                nc.vector.tensor_mul(t, ps_bc[j][:, 0:1], ab_c[j][0])
                ab_c[j][1] = t  # temp
            elif step == 9:
                b_c = work.tile([P, 1], F32, tag="b_c", name=f"b_c{j}")
                nc.vector.tensor_sub(b_c, bias_sb[j], ab_c[j][1])
                ab_c[j][1] = b_c

        def xn_piece(j, pc, xpc):
            a_c, b_c = ab_c[j]
            sl = slice(pc * (n // xpc), (pc + 1) * (n // xpc))
            if pc % 2 == 0:
                nc.vector.tensor_scalar(
                    out=xn_bf[:, j, sl], in0=x_sb[j][:, sl],
                    scalar1=a_c, scalar2=b_c,
                    op0=mybir.AluOpType.mult, op1=mybir.AluOpType.add)
            else:
                nc.scalar.activation(
                    out=xn_bf[:, j, sl], in_=x_sb[j][:, sl],
                    func=mybir.ActivationFunctionType.Identity,
                    bias=b_c, scale=a_c)
            nc.gpsimd.tensor_scalar(
                out=xn8[:, j, sl], in0=x_sb[j][:, sl],
                scalar1=a_c, scalar2=b_c,
                op0=mybir.AluOpType.mult, op1=mybir.AluOpType.add)

        # stats per chunk, then both chunks' chains stepwise-interleaved
        # (sequential GN psum lifetimes: a deferred chain holds pool tiles
        # that gate the projection bank rotation - measured +27us head)
        for j in range(CK):
            for s in range(npc):
                nc.vector.bn_stats(out=stats[j][:, s, :], in_=xr[j][:, s, :])
            nc.vector.bn_aggr(out=mv2[j], in_=stats[j])
        for step in range(10):
            for j in range(CK):
                gn_chain(j, step)
        xpc = max(1, n // 1024)
        for pc in range(xpc):
            for j in range(CK):
                xn_piece(j, pc, xpc)

        # ---------------- projections (pre-attention) ----------------
        # The attention steady state is PE-limited at ~1.01us/pair-slot vs
        # ACT ~1.0us, so there is no PE slack to hide projection matmuls
        # inside the stream - they run up front. q/k in bf16 (score-path
        # precision drives the final max error), vtw in fp8 DoubleRow.
        # PSUM evictions alternate DVE/ACT here (ACT is idle pre-exp);
        # vtw evictions are DVE-only and trail into the attention phase.
        # All projection psum comes from ps_a only: ps_s must stay free for
        # the S stream (any projection tile in its rotation delays the
        # stream start behind the whole projection phase).
        def proj_ps(shape):
            return ps_a.tile(shape, F32, tag="a", name="ps_p")

        # projection evictions alternate DVE/ACT per tile (both engines can
        # read PSUM; ACT is idle pre-exp). A single-engine eviction stream
        # saturates and the MM->evict->MM bank chains stall the PE ~19us.
        evx = [0]

        def emit_qk(idx, nb, in_stream=False):
            dst = q3 if idx == 0 else k3
            nsl = slice(nb * NB, (nb + 1) * NB)
            for oc in range(CK):
                o_off = idx * C + oc * P
                ps = proj_ps([P, NB])
                for kc in range(CK):
                    nc.tensor.matmul(
                        ps,
                        wqkT_b[:, kc, o_off:o_off + P],
                        xn_bf[:, kc, nsl],
                        start=(kc == 0), stop=(kc == CK - 1),
                    )
                evx[0] ^= 1
                if evx[0]:
                    nc.vector.tensor_scalar_add(
                        out=dst[:, oc, nsl], in0=ps,
                        scalar1=bqk_sb[idx * CK + oc])
                else:
                    nc.scalar.activation(
                        out=dst[:, oc, nsl], in_=ps,
                        func=mybir.ActivationFunctionType.Identity,
                        bias=bqk_sb[idx * CK + oc], scale=1.0)

        # Pre-stream order tuned for the DVE eviction FIFO: k + q0 first
        # (gate the S stream), then vtw, then q(1..7) - so finalize(0),
        # emitted later, is reached on DVE before the q(1..7) evictions
        # would otherwise delay it (q(b) is only needed at stream slot 16b).
        for nb in range(nblk):
            emit_qk(1, nb)
        emit_qk(0, 0)
        for mt in range(mtn):
            ps = proj_ps([P, C])
            nc.tensor.matmul(
                ps,
                xn8[:, :, mt * P:(mt + 1) * P],
                wvoT_8,
                perf_mode=mybir.MatmulPerfMode.DoubleRow,
                start=True, stop=True,
            )
            evx[0] ^= 1
            if evx[0]:
                nc.vector.tensor_copy(vtw_sb[:, mt, 0:C], ps)
            else:
                nc.scalar.activation(
                    out=vtw_sb[:, mt, 0:C], in_=ps,
                    func=mybir.ActivationFunctionType.Identity,
                    bias=0.0, scale=1.0)
        for nb in range(1, nblk):
            emit_qk(0, nb)

        # preload the exp table set (sqrt set was loaded during GroupNorm);
        # emitted after the last pre-stream ACT instruction so the table
        # load lands here instead of sinking into the exp stream
        dummy_exp = work.tile([1, 1], F32, tag="dummy", name="dummy_exp")
        nc.scalar.activation(out=dummy_exp, in_=eps_sb[0:1, :],
                             func=mybir.ActivationFunctionType.Exp)

        # ---------------- attention emitters ----------------
        DR = mybir.MatmulPerfMode.DoubleRow

        def emit_s_pair(blk, p):
            nsl = slice(blk * NB, (blk + 1) * NB)
            ps = ps_s.tile([P, 2, NB], F32, tag="s", name="ps_s")
            for sub in range(2):
                mt = 2 * p + sub
                nc.tensor.matmul(
                    ps[:, sub, :],
                    k3[:, :, mt * P:(mt + 1) * P],
                    q3[:, :, nsl],
                    perf_mode=DR, start=True, stop=True,
                )
            nc.scalar.activation(
                out=expT[blk % 2][:, 2 * p:2 * p + 2, :], in_=ps,
                func=mybir.ActivationFunctionType.Exp,
                bias=eln_sb,
            )

        def emit_ao_pair(ao_ps, blk, half, j):
            # stream only the 257 meaningful columns of the 272-padded vtw
            # (the DR middle-step constraint is on the storage stride, 272)
            for c in (half * 2, half * 2 + 1):
                nc.tensor.matmul(
                    ao_ps[c - half * 2][:, 0:AOF],
                    expT[blk % 2][:, 2 * j:2 * j + 2, c * P:(c + 1) * P],
                    vtw_sb[:, 2 * j:2 * j + 2, 0:AOF],
                    perf_mode=DR,
                    start=(j == 0), stop=(j == npairs - 1),
                )

        def finalize(blk, half, ao_ps):
            for i, c in enumerate((half * 2, half * 2 + 1)):
                ps = ao_ps[i]
                recip = work.tile([P, 1], F32, tag="recip", name="recip")
                nc.vector.reciprocal(recip, ps[:, C:C + 1])
                o_sb = evac.tile([P, C], F32, tag="o_sb", name="o_sb")
                nc.vector.scalar_tensor_tensor(
                    out=o_sb, in0=ps[:, 0:C], scalar=recip,
                    in1=xtb_sb[:, blk * NSUB + c, :],
                    op0=mybir.AluOpType.mult, op1=mybir.AluOpType.add,
                )
                row0 = (blk * NSUB + c) * P
                nc.sync.dma_start(out=out_d[row0:row0 + P, :], in_=o_sb)

        # ---------------- flat attention pipeline ----------------
        # Half-lag AO: q-subs {0,1} of block b accumulate during block b's
        # S slots; subs {2,3} run one full block later against the
        # double-buffered expT. Each 2-bank half then has a whole block of
        # slack for its finalize to release banks - no boundary stalls.
        total_pairs = nblk * npairs
        ao_tiles = {}   # (blk, half) -> 2 psum tiles
        for gp in range(total_pairs + npairs + LAG):
            if gp < total_pairs:
                emit_s_pair(gp // npairs, gp % npairs)
            for half in range(2):
                ag = gp - LAG - half * npairs
                if ag < 0:
                    continue
                ablk, aj = divmod(ag, npairs)
                if ablk >= nblk:
                    continue
                key = (ablk, half)
                if aj == 0:
                    ao_tiles[key] = [
                        ps_a.tile([P, C + VPAD], F32, tag="a",
                                  name=f"ps_ao{half}{c}")
                        for c in range(2)
                    ]
                emit_ao_pair(ao_tiles[key], ablk, half, aj)
                if aj == npairs - 1:
                    finalize(ablk, half, ao_tiles.pop(key))

    return nc


_CACHED_NC = {}


def build_nc(n=N):
    if n not in _CACHED_NC:
        nc = bacc.Bacc("TRN2", target_bir_lowering=False, debug=False,
                       num_devices=B)
        build_attention_program(nc, n=n)
        nc.compile()
        _CACHED_NC[n] = nc
    return _CACHED_NC[n]


def make_in_maps(x, gn_scale, gn_bias, w_qkv, b_qkv, w_out, b_out):
    f = np.ascontiguousarray
    r = float(np.sqrt(SCALE))
    wq = w_qkv[:C] * r
    wk = w_qkv[C:2 * C] * r
    wqk = np.concatenate([wq, wk], axis=0)            # [512, 256]
    wqkT = wqk.T.reshape(CK, P, O_QK).transpose(1, 0, 2)   # [128, 2, 512]
    wvo = w_out @ w_qkv[2 * C:]                        # [256, 256]
    wvoT = wvo.T.reshape(CK, P, C).transpose(1, 0, 2)  # [128, 2, 256]
    bqk = np.concatenate([b_qkv[:C] * r, b_qkv[C:2 * C] * r])
    bvo = w_out @ b_qkv[2 * C:]
    add_c = (b_out + bvo).astype(np.float32)           # [256]
    return [
        {
            "x": f(x[b].reshape(C, N), dtype=np.float32),
            "xtb": f(x[b].reshape(C, N).T + add_c[None, :], dtype=np.float32),
            "wqkT": f(wqkT, dtype=np.float32),
            "wvoT": f(wvoT, dtype=np.float32),
            "gn_scale": f(gn_scale, dtype=np.float32),
            "gn_bias": f(gn_bias, dtype=np.float32),
            "bqk": f(bqk, dtype=np.float32),
        }
        for b in range(B)
    ]


def kernel(x, gn_scale, gn_bias, w_qkv, b_qkv, w_out, b_out, _trace=False,
           _tmpdir=None):
    x = np.asarray(x)
    gn_scale = np.asarray(gn_scale)
    gn_bias = np.asarray(gn_bias)
    w_qkv = np.asarray(w_qkv)
    b_qkv = np.asarray(b_qkv)
    w_out = np.asarray(w_out)
    b_out = np.asarray(b_out)
    nc = build_nc()
    in_maps = make_in_maps(x, gn_scale, gn_bias, w_qkv, b_qkv, w_out, b_out)
    res = run_bass_kernel_spmd(nc, in_maps, list(range(B)), trace=_trace,
                               tmpdir=_tmpdir)
    out = np.stack([res.results[b]["out"].T for b in range(B)])
    out = out.reshape(B, C, H, W).astype(np.float32)
    if _trace:
        kernel.last_exec_time_ns = res.exec_time_ns
        kernel.last_results = res
    return out
